# revision 1
# baseline (speedup 1.0000x reference)
"""MoE (top-2 of 8 experts, silu MLP 1024->4096->1024) on 8 Trainium2 cores.

Strategy: expert-parallel. The tiny router runs on host (fp32, exact same
math as the reference); tokens are dispatched (gathered) per expert on the
host — this is the "all-to-all" — and each NeuronCore runs one expert's
2-layer MLP over its token batch, with the top-k combine weight applied
on-device. The host scatter-adds the per-expert results into the full
output. Matmuls use float32r (TF32-like, full PE rate, ~1e-4 rel err)
with fp32 PSUM accumulation.

Per-core kernel loop: i-blocks of 512 inter channels (weights streamed
once, double-buffered) x token blocks of 384. MLP1 computes
hT = silu(W1_blk.T @ xT) per 128-wide inter chunk on the fly; MLP2
accumulates y[tok, hid] in 6 PSUM banks across the i-block's 4 chunks,
then DVE drains into an SBUF accumulator. The last i-block applies the
per-token scale (ScalarE) and DMAs out.
"""
import numpy as np

HIDDEN = 1024
INTER = 4096
TOP_K = 2
TB = 384          # tokens per block (3 chunks of 128)
IB = 512          # inter channels per i-block (4 chunks of 128)

_CACHE = {}


def _build(C: int, loop_iters: int = 1):
    """Build + compile the per-core expert-MLP kernel for capacity C."""
    import concourse.bacc as bacc
    import concourse.mybir as mybir
    import concourse.tile as tile

    dt = mybir.dt
    AF = mybir.ActivationFunctionType
    assert C % TB == 0
    nTB = C // TB
    n_ib = INTER // IB
    n_ic = IB // 128
    n_kh = HIDDEN // 128
    n_tc = TB // 128
    n_hh = HIDDEN // 512

    nc = bacc.Bacc("TRN2", target_bir_lowering=False, debug=False)
    f32, f32r = dt.float32, dt.float32r

    xT_d = nc.dram_tensor("xt", [HIDDEN, C], f32r, kind="ExternalInput")
    w1_d = nc.dram_tensor("w1", [HIDDEN, INTER], f32r, kind="ExternalInput")
    w2_d = nc.dram_tensor("w2", [INTER, HIDDEN], f32r, kind="ExternalInput")
    sc_d = nc.dram_tensor("sc", [C], f32, kind="ExternalInput")
    y_d = nc.dram_tensor("y", [C, HIDDEN], f32, kind="ExternalOutput")

    with tile.TileContext(nc) as tc:
        with (
            tc.tile_pool(name="wp", bufs=2) as wp,
            tc.tile_pool(name="xp", bufs=2) as xp,
            tc.tile_pool(name="hp", bufs=3) as hp,
            tc.tile_pool(name="yacc", bufs=1) as yp,
            tc.tile_pool(name="scp", bufs=1) as scp,
            tc.tile_pool(name="outp", bufs=3) as op,
            tc.tile_pool(name="ph", bufs=2, space="PSUM") as ph,
            tc.tile_pool(name="py", bufs=1, space="PSUM") as py,
        ):
            # scale laid out [128, C//128]: column t = tokens t*128..t*128+127
            sc = scp.tile([128, C // 128], f32, tag="sc", name="sc")
            nc.sync.dma_start(sc[:], sc_d.ap().rearrange("(n p) -> p n", p=128))

            def body():
                for ib in range(n_ib):
                    w1t = []  # [128h, IB] per hidden chunk
                    for kh in range(n_kh):
                        t = wp.tile([128, IB], f32r, tag=f"w1_{kh}",
                                    name=f"w1_{kh}")
                        nc.sync.dma_start(
                            t[:], w1_d.ap()[kh * 128:(kh + 1) * 128,
                                            ib * IB:(ib + 1) * IB])
                        w1t.append(t)
                    w2t = []  # [128i, HIDDEN] per inter chunk
                    for ic in range(n_ic):
                        t = wp.tile([128, HIDDEN], f32r, tag=f"w2_{ic}",
                                    name=f"w2_{ic}")
                        nc.sync.dma_start(
                            t[:], w2_d.ap()[ib * IB + ic * 128:
                                            ib * IB + (ic + 1) * 128, :])
                        w2t.append(t)

                    for tb in range(nTB):
                        xb = []
                        for kh in range(n_kh):
                            t = xp.tile([128, TB], f32r, tag=f"x_{kh}",
                                        name=f"x_{kh}")
                            nc.sync.dma_start(
                                t[:], xT_d.ap()[kh * 128:(kh + 1) * 128,
                                                tb * TB:(tb + 1) * TB])
                            xb.append(t)

                        y_ps = [py.tile([128, 512], f32, tag=f"y{t_}_{hh}",
                                        name=f"y{t_}_{hh}")
                                for t_ in range(n_tc) for hh in range(n_hh)]

                        for ic in range(n_ic):
                            p_h = ph.tile([128, TB], f32, tag="h", name="p_h")
                            for kh in range(n_kh):
                                nc.tensor.matmul(
                                    p_h[:],
                                    w1t[kh][:, ic * 128:(ic + 1) * 128],
                                    xb[kh][:],
                                    start=(kh == 0), stop=(kh == n_kh - 1))
                            h2 = hp.tile([128, TB], f32r, tag="h2", name="h2")
                            nc.scalar.activation(h2[:], p_h[:], AF.Silu)
                            for t_ in range(n_tc):
                                for hh in range(n_hh):
                                    nc.tensor.matmul(
                                        y_ps[t_ * n_hh + hh][:],
                                        h2[:, t_ * 128:(t_ + 1) * 128],
                                        w2t[ic][:, hh * 512:(hh + 1) * 512],
                                        start=(ic == 0), stop=(ic == n_ic - 1))

                        for t_ in range(n_tc):
                            ya = yp.tile([128, HIDDEN], f32,
                                         tag=f"ya_{tb}_{t_}",
                                         name=f"ya_{tb}_{t_}")
                            tok = tb * n_tc + t_
                            for hh in range(n_hh):
                                p = y_ps[t_ * n_hh + hh][:]
                                dst = ya[:, hh * 512:(hh + 1) * 512]
                                if ib == 0:
                                    nc.vector.tensor_copy(dst, p)
                                else:
                                    nc.vector.tensor_add(dst, dst, p)
                            if ib == n_ib - 1:
                                out = op.tile([128, HIDDEN], f32, tag="out",
                                              name="out")
                                nc.scalar.activation(
                                    out[:], ya[:], AF.Copy,
                                    scale=sc[:, tok:tok + 1])
                                nc.sync.dma_start(
                                    y_d.ap()[tok * 128:(tok + 1) * 128, :],
                                    out[:])

            if loop_iters == 1:
                body()
            else:
                with tc.For_i(0, loop_iters, 1):
                    body()
    nc.compile()
    return nc


def _route(x, router_w):
    """Host router: softmax -> top-2 -> renormalize, plus aux loss.

    Mirrors the reference math in fp32 numpy.
    """
    T = x.shape[0]
    E = router_w.shape[0]
    logits = x @ router_w.T                                   # [T, E]
    m = logits.max(axis=-1, keepdims=True)
    ex = np.exp(logits - m)
    probs = ex / ex.sum(axis=-1, keepdims=True)
    ar = np.arange(T)
    top1 = probs.argmax(axis=-1)
    p1 = probs[ar, top1]
    probs_m = probs.copy()
    probs_m[ar, top1] = -np.inf
    top2 = probs_m.argmax(axis=-1)
    p2 = probs[ar, top2]
    s = p1 + p2
    v1, v2 = p1 / s, p2 / s

    tokens_per_expert = np.bincount(top1, minlength=E).astype(np.float32)
    router_prob = probs.mean(axis=0)
    aux_loss = np.float32(
        np.sum(tokens_per_expert / np.float32(T) * router_prob) * E)
    return top1, top2, v1.astype(np.float32), v2.astype(np.float32), aux_loss


def _pack(x, top1, top2, v1, v2, n_experts):
    """Gather tokens per expert, pad to a uniform capacity C."""
    T, H = x.shape
    idxs, scales = [], []
    for e in range(n_experts):
        i1 = np.nonzero(top1 == e)[0]
        i2 = np.nonzero(top2 == e)[0]
        idx = np.concatenate([i1, i2])
        w = np.concatenate([v1[i1], v2[i2]])
        idxs.append(idx)
        scales.append(w)
    max_n = max(len(i) for i in idxs)
    C = max(TB, ((max_n + TB - 1) // TB) * TB)
    in_maps = []
    for e in range(n_experts):
        idx, w = idxs[e], scales[e]
        xt = np.zeros((H, C), dtype=np.float32)
        xt[:, :len(idx)] = x[idx].T
        sc = np.zeros(C, dtype=np.float32)
        sc[:len(idx)] = w
        in_maps.append({"xt": xt, "sc": sc})
    return idxs, in_maps, C


def kernel(x, router_w, expert_w1, expert_w2):
    from concourse import bass_utils

    x = np.ascontiguousarray(np.asarray(x, dtype=np.float32))
    router_w = np.asarray(router_w, dtype=np.float32)
    expert_w1 = np.asarray(expert_w1, dtype=np.float32)
    expert_w2 = np.asarray(expert_w2, dtype=np.float32)
    T, H = x.shape
    E = router_w.shape[0]

    top1, top2, v1, v2, aux_loss = _route(x, router_w)
    idxs, in_maps, C = _pack(x, top1, top2, v1, v2, E)
    for e in range(E):
        in_maps[e]["w1"] = np.ascontiguousarray(expert_w1[e])
        in_maps[e]["w2"] = np.ascontiguousarray(expert_w2[e])

    if C not in _CACHE:
        _CACHE[C] = _build(C)
    nc = _CACHE[C]

    res = bass_utils.run_bass_kernel_spmd(nc, in_maps, core_ids=list(range(E)))

    output = np.zeros((T, H), dtype=np.float32)
    for e in range(E):
        idx = idxs[e]
        output[idx] += res.results[e]["y"][:len(idx)]
    return output, aux_loss


# revision 2
# speedup vs baseline: 1.4830x; 1.4830x over previous
"""MoE (top-2 of 8 experts, silu MLP 1024->4096->1024) on 8 Trainium2 cores.

Strategy: expert-parallel. The tiny router runs on host (fp32, exact same
math as the reference); tokens are dispatched (gathered) per expert on the
host — this is the "all-to-all" — and each NeuronCore runs one expert's
2-layer MLP over its token batch, with the top-k combine weight applied
on-device. The host scatter-adds the per-expert results into the full
output. Matmuls use float32r (TF32-like, full PE rate, ~1e-4 rel err)
with fp32 PSUM accumulation.

Per-core kernel loop: i-blocks of 512 inter channels (weights streamed
once, double-buffered) x token blocks of <=384. MLP1 computes
hT = silu(W1_blk.T @ xT) per 128-wide inter chunk on the fly; MLP2
accumulates y[tok, hid] in up to 6 PSUM banks across the i-block's 4
chunks, then DVE drains into an SBUF accumulator. The last i-block
applies the per-token scale (ScalarE) and DMAs out.
"""
import numpy as np

HIDDEN = 1024
INTER = 4096
TOP_K = 2
TB = 384          # max tokens per block (3 chunks of 128)
IB = 512          # inter channels per i-block (4 chunks of 128)

_CACHE = {}


def _blocks_for(max_n: int):
    """Token-block sizes covering max_n: full 384s plus a 256 remainder
    when possible (fp32r needs moving dim >= 256 for full PE rate)."""
    n_full = max_n // TB
    rem = max_n - n_full * TB
    blocks = [TB] * n_full
    if rem > 0:
        blocks.append(256 if rem <= 256 else TB)
    if not blocks:
        blocks = [256]
    return blocks


def _build(blocks, loop_iters: int = 1):
    """Build + compile the per-core expert-MLP kernel for token blocks."""
    import concourse.bacc as bacc
    import concourse.mybir as mybir
    import concourse.tile as tile

    dt = mybir.dt
    AF = mybir.ActivationFunctionType
    C = sum(blocks)
    assert C % 128 == 0
    starts = [sum(blocks[:i]) for i in range(len(blocks))]
    nTB = len(blocks)
    n_ib = INTER // IB
    n_ic = IB // 128
    n_kh = HIDDEN // 128
    n_tc_max = TB // 128
    n_hh = HIDDEN // 512

    nc = bacc.Bacc("TRN2", target_bir_lowering=False, debug=False)
    f32, f32r = dt.float32, dt.float32r

    xT_d = nc.dram_tensor("xt", [HIDDEN, C], f32r, kind="ExternalInput")
    w1_d = nc.dram_tensor("w1", [HIDDEN, INTER], f32r, kind="ExternalInput")
    w2_d = nc.dram_tensor("w2", [INTER, HIDDEN], f32r, kind="ExternalInput")
    sc_d = nc.dram_tensor("sc", [C], f32, kind="ExternalInput")
    y_d = nc.dram_tensor("y", [C, HIDDEN], f32, kind="ExternalOutput")

    with tile.TileContext(nc) as tc:
        with (
            tc.tile_pool(name="wp", bufs=2) as wp,
            tc.tile_pool(name="xp", bufs=2) as xp,
            tc.tile_pool(name="hp", bufs=3) as hp,
            tc.tile_pool(name="yacc", bufs=1) as yp,
            tc.tile_pool(name="scp", bufs=1) as scp,
            tc.tile_pool(name="outp", bufs=3) as op,
            tc.tile_pool(name="ph", bufs=2, space="PSUM") as ph,
            tc.tile_pool(name="py", bufs=1, space="PSUM") as py,
        ):
            # scale laid out [128, C//128]: column t = tokens t*128..t*128+127
            sc = scp.tile([128, C // 128], f32, tag="sc", name="sc")
            nc.sync.dma_start(sc[:], sc_d.ap().rearrange("(n p) -> p n", p=128))

            def body():
                for ib in range(n_ib):
                    w1t = []  # [128h, IB] per hidden chunk
                    for kh in range(n_kh):
                        t = wp.tile([128, IB], f32r, tag=f"w1_{kh}",
                                    name=f"w1_{kh}")
                        nc.sync.dma_start(
                            t[:], w1_d.ap()[kh * 128:(kh + 1) * 128,
                                            ib * IB:(ib + 1) * IB])
                        w1t.append(t)
                    w2t = []  # [128i, HIDDEN] per inter chunk
                    for ic in range(n_ic):
                        t = wp.tile([128, HIDDEN], f32r, tag=f"w2_{ic}",
                                    name=f"w2_{ic}")
                        nc.sync.dma_start(
                            t[:], w2_d.ap()[ib * IB + ic * 128:
                                            ib * IB + (ic + 1) * 128, :])
                        w2t.append(t)

                    for tb in range(nTB):
                        TBS = blocks[tb]
                        t0_tok = starts[tb]
                        n_tc = TBS // 128
                        xb = []
                        for kh in range(n_kh):
                            t = xp.tile([128, TB], f32r, tag=f"x_{kh}",
                                        name=f"x_{kh}")
                            nc.sync.dma_start(
                                t[:, :TBS],
                                xT_d.ap()[kh * 128:(kh + 1) * 128,
                                          t0_tok:t0_tok + TBS])
                            xb.append(t)

                        y_ps = [py.tile([128, 512], f32, tag=f"y{t_}_{hh}",
                                        name=f"y{t_}_{hh}")
                                for t_ in range(n_tc_max) for hh in range(n_hh)]

                        for ic in range(n_ic):
                            p_h = ph.tile([128, TB], f32, tag="h", name="p_h")
                            for kh in range(n_kh):
                                nc.tensor.matmul(
                                    p_h[:, :TBS],
                                    w1t[kh][:, ic * 128:(ic + 1) * 128],
                                    xb[kh][:, :TBS],
                                    start=(kh == 0), stop=(kh == n_kh - 1))
                            h2 = hp.tile([128, TB], f32r, tag="h2", name="h2")
                            nc.scalar.activation(h2[:, :TBS], p_h[:, :TBS],
                                                 AF.Silu)
                            for t_ in range(n_tc):
                                for hh in range(n_hh):
                                    nc.tensor.matmul(
                                        y_ps[t_ * n_hh + hh][:],
                                        h2[:, t_ * 128:(t_ + 1) * 128],
                                        w2t[ic][:, hh * 512:(hh + 1) * 512],
                                        start=(ic == 0), stop=(ic == n_ic - 1))

                        for t_ in range(n_tc):
                            ya = yp.tile([128, HIDDEN], f32,
                                         tag=f"ya_{tb}_{t_}",
                                         name=f"ya_{tb}_{t_}")
                            tok = t0_tok // 128 + t_
                            for hh in range(n_hh):
                                p = y_ps[t_ * n_hh + hh][:]
                                dst = ya[:, hh * 512:(hh + 1) * 512]
                                if ib == 0:
                                    nc.vector.tensor_copy(dst, p)
                                else:
                                    nc.vector.tensor_add(dst, dst, p)
                            if ib == n_ib - 1:
                                out = op.tile([128, HIDDEN], f32, tag="out",
                                              name="out")
                                nc.scalar.activation(
                                    out[:], ya[:], AF.Copy,
                                    scale=sc[:, tok:tok + 1])
                                nc.sync.dma_start(
                                    y_d.ap()[tok * 128:(tok + 1) * 128, :],
                                    out[:])

            if loop_iters == 1:
                body()
            else:
                with tc.For_i(0, loop_iters, 1):
                    body()
    nc.compile()
    return nc


def _route(x, router_w):
    """Host router: softmax -> top-2 -> renormalize, plus aux loss.

    Mirrors the reference math in fp32 numpy.
    """
    T = x.shape[0]
    E = router_w.shape[0]
    logits = x @ router_w.T                                   # [T, E]
    m = logits.max(axis=-1, keepdims=True)
    ex = np.exp(logits - m)
    probs = ex / ex.sum(axis=-1, keepdims=True)
    ar = np.arange(T)
    top1 = probs.argmax(axis=-1)
    p1 = probs[ar, top1]
    probs_m = probs.copy()
    probs_m[ar, top1] = -np.inf
    top2 = probs_m.argmax(axis=-1)
    p2 = probs[ar, top2]
    s = p1 + p2
    v1, v2 = p1 / s, p2 / s

    tokens_per_expert = np.bincount(top1, minlength=E).astype(np.float32)
    router_prob = probs.mean(axis=0)
    aux_loss = np.float32(
        np.sum(tokens_per_expert / np.float32(T) * router_prob) * E)
    return top1, top2, v1.astype(np.float32), v2.astype(np.float32), aux_loss


def _pack(x, top1, top2, v1, v2, n_experts):
    """Gather tokens per expert, pad to a uniform capacity."""
    T, H = x.shape
    idxs, scales = [], []
    for e in range(n_experts):
        i1 = np.nonzero(top1 == e)[0]
        i2 = np.nonzero(top2 == e)[0]
        idxs.append(np.concatenate([i1, i2]))
        scales.append(np.concatenate([v1[i1], v2[i2]]))
    max_n = max(len(i) for i in idxs)
    blocks = _blocks_for(max_n)
    C = sum(blocks)
    in_maps = []
    for e in range(n_experts):
        idx, w = idxs[e], scales[e]
        xt = np.zeros((H, C), dtype=np.float32)
        xt[:, :len(idx)] = x[idx].T
        sc = np.zeros(C, dtype=np.float32)
        sc[:len(idx)] = w
        in_maps.append({"xt": xt, "sc": sc})
    return idxs, in_maps, blocks


def kernel(x, router_w, expert_w1, expert_w2):
    from concourse import bass_utils

    x = np.ascontiguousarray(np.asarray(x, dtype=np.float32))
    router_w = np.asarray(router_w, dtype=np.float32)
    expert_w1 = np.asarray(expert_w1, dtype=np.float32)
    expert_w2 = np.asarray(expert_w2, dtype=np.float32)
    T, H = x.shape
    E = router_w.shape[0]

    top1, top2, v1, v2, aux_loss = _route(x, router_w)
    idxs, in_maps, blocks = _pack(x, top1, top2, v1, v2, E)
    for e in range(E):
        in_maps[e]["w1"] = np.ascontiguousarray(expert_w1[e])
        in_maps[e]["w2"] = np.ascontiguousarray(expert_w2[e])

    key = tuple(blocks)
    if key not in _CACHE:
        _CACHE[key] = _build(blocks)
    nc = _CACHE[key]

    res = bass_utils.run_bass_kernel_spmd(nc, in_maps, core_ids=list(range(E)))

    output = np.zeros((T, H), dtype=np.float32)
    for e in range(E):
        idx = idxs[e]
        output[idx] += res.results[e]["y"][:len(idx)]
    return output, aux_loss
